# revision 7
# baseline (speedup 1.0000x reference)
"""Trainium2 Bass kernel for nn_Attention_39676907884025.

out[b, q, :] = (1/SK) * sum_k value[b, k, :] for every q: q_param (1x1) is
broadcast over query and key, the score matrix is constant along the softmax
axis, and softmax of a constant row is exactly uniform. Only `value` touches
the device; batch B=16 is data-parallel over 8 cores (2 per core).

Raw bacc, hand-scheduled, NO nc.Block. Rationale (from perfetto traces of
the previous versions):
  - The NEFF epilogue makes every engine serially reset its fixed bank of
    ~50 semaphores (0.05-0.13 us each => 2.2-6.5 us per engine). With
    nc.Block, its exit barrier forces all engines to finish the body first,
    so the slowest reset chain lands entirely after the last store
    (~8 us of pure postamble). Emitting raw per-engine streams (no block,
    no exit barrier) lets each engine start its resets right after its own
    last instruction, overlapping them with the DMA tail. The NEFF's own
    final all-engine barrier before the loop-back jump still serializes
    executions, and the entry barrier isolates re-runs.
  - This requires bank-aware semaphore placement: an engine resets its bank
    whenever IT finishes, so a semaphore may only live in bank X if its
    last wait/increment is causally ordered before engine X's last body
    instruction. Banks: PE S[7:54], ACT S[54:105], Pool S[105:156],
    DVE S[156:207], SP S[207:256]; the user pool starts at 155 (Pool bank
    tail - burn it, Pool's body is empty and it resets almost at t=0).
  - All DMA on one HWDGE queue (SP): dma_start issue cost (~0.6 us) pays a
    SHARED HWDGE unit, so spreading across engines doesn't parallelize it,
    and a single queue still fans out over all 16 SDMA engines at full
    HBM rate while making chunk completion strictly FIFO (better
    pipelining than 2 queues' round-robin). 4 load chunks per batch
    (256 KB, 2 KB descriptors) for reduce overlap; ONE store per batch
    via a stride-0 broadcast source AP (1024 x 512 B descriptors) so the
    mean tile only needs 2 replicas in SBUF.
  - DVE pairwise-adds each chunk (128,512)f32 -> (128,256)bf16 as it
    lands; PE accumulates the 4 bf16 blocks per batch into a (128,256)
    fp32 PSUM tile with a constant 1/SK stationary (partition-reduce +
    broadcast); DVE folds psum halves -> (128,256) bf16 wide tile (two
    independent adds). ACT's only job is relaying the PE semaphore to DVE
    (only ACT may wait on PE sems - other engines hang the device).
  - Stores in bf16 (host upcasts; mean error ~0.3% << 2e-2 budget).
"""

import sys

import numpy as np

if "/opt/trn_rl_repo" not in sys.path:
    sys.path.insert(0, "/opt/trn_rl_repo")

B, SQ, SK, D, DV = 16, 2048, 2048, 128, 128
N_CORES = 8
BPC = B // N_CORES  # batches per core
P = 128

LAST_RESULT = None  # BassKernelResults of the most recent run (for profiling)


def _build_nc():
    import concourse.bacc as bacc
    import concourse.mybir as mybir

    f32 = mybir.dt.float32
    bf16 = mybir.dt.bfloat16
    nc = bacc.Bacc("TRN2", target_bir_lowering=False)

    val = nc.dram_tensor("value", [BPC, SK, DV], f32, kind="ExternalInput")
    out = nc.dram_tensor("out", [BPC, SQ, DV], bf16, kind="ExternalOutput")

    w = nc.alloc_sbuf_tensor("w_const", [P, P], bf16)
    xts = [nc.alloc_sbuf_tensor(f"xt{b}", [P, SK], f32) for b in range(BPC)]
    # pairwise sums per chunk c: bf16 at [256c, 256c+256)
    lv1 = [nc.alloc_sbuf_tensor(f"lv1_{b}", [P, 1024], bf16) for b in range(BPC)]
    # two replicas of the folded bf16 mean row
    wide = [nc.alloc_sbuf_tensor(f"wide{b}", [P, 256], bf16) for b in range(BPC)]
    pss = [nc.alloc_psum_tensor(f"ps{b}", [P, P], f32) for b in range(BPC)]

    # --- bank-aware semaphore allocation (pool pops 155, 156, ... in order)
    def sem(name, expect):
        s = nc.alloc_semaphore(name)
        assert s.num == expect, (name, s.num, expect)
        return s

    sem("dummy_pool_bank", 155)  # Pool resets S[155] at ~t=0; never use it
    # DVE bank S[156:207]: last waits/incs all causally precede DVE's last
    # fold (s_ld/s_rel: DVE's own waits; s_w/s_dve: PE consumes before its
    # stop-matmul -> s_mm -> relay -> DVE fold; s_mm: ACT consumes before
    # relay -> DVE fold).
    s_ld = [[sem(f"s_ld_{b}_{c}", 156 + 4 * b + c) for c in range(4)] for b in range(BPC)]
    s_w = sem("s_w", 164)
    s_dve = [sem(f"s_dve_{b}", 165 + b) for b in range(BPC)]
    s_mm = sem("s_mm", 167)
    s_rel = sem("s_rel", 168)
    for i in range(169, 207):  # burn the rest of the DVE bank
        sem(f"dummy_{i}", i)
    # SP bank S[207:256]: SP's own final waits consume these
    s_wide = [sem(f"s_wide_{b}", 207 + b) for b in range(BPC)]
    s_st = sem("s_st", 209)

    def xdst(b):
        return xts[b][:].rearrange("p (t d) -> p t d", d=DV)

    def xsrc(b):
        return val[b].rearrange("(p t) d -> p t d", p=P)

    # chunk boundaries in t-groups (of 16 rows); b1's tail chunks are small
    # so the last-chunk -> L1 -> matmul -> store chain is short
    CHUNKS = [(0, 4, 8, 12, 16), (0, 6, 12, 14, 16)]

    def load(eng, b, c):
        t0, t1 = CHUNKS[b][c], CHUNKS[b][c + 1]
        eng.dma_start(
            xdst(b)[:, t0:t1, :], xsrc(b)[:, t0:t1, :]
        ).then_inc(s_ld[b][c], 16)

    # --- loads: batch-0 chunks lead on both queues; Scalar issues first
    # (SP's entry DRAIN costs ~0.7 us, ACT's is instant)
    load(nc.scalar, 0, 0)
    load(nc.sync, 0, 1)
    load(nc.scalar, 0, 2)
    load(nc.sync, 0, 3)
    load(nc.scalar, 1, 0)
    load(nc.sync, 1, 1)
    load(nc.scalar, 1, 2)
    load(nc.sync, 1, 3)

    # --- SP: store batch 0, then the final completion wait for both stores
    nc.sync.wait_ge(s_wide[0], 1)
    nc.sync.dma_start(
        out[0].rearrange("(p t u) d -> p t (u d)", p=P, t=8),
        wide[0][:][:, None, :].to_broadcast((P, 8, 256)),
    ).then_inc(s_st, 16)
    nc.sync.wait_ge(s_st, 32)

    # --- ACT: sole waiter on the PE semaphore; relays to DVE; stores batch 1
    nc.scalar.wait_ge(s_mm, 1)
    nc.scalar.sem_inc(s_rel, 1)
    nc.scalar.wait_ge(s_mm, 2)
    nc.scalar.sem_inc(s_rel, 1)
    nc.scalar.wait_ge(s_wide[1], 1)
    nc.scalar.dma_start(
        out[1].rearrange("(p t u) d -> p t (u d)", p=P, t=8),
        wide[1][:][:, None, :].to_broadcast((P, 8, 256)),
    ).then_inc(s_st, 16)

    # --- DVE: L1 pairwise adds (f32 -> bf16) + psum widen into wide
    nc.vector.memset(w[:], 1.0 / SK).then_inc(s_w, 1)
    for b in range(BPC):
        for c in range(4):
            t0, t1 = CHUNKS[b][c], CHUNKS[b][c + 1]
            lo, half = 128 * t0, 64 * (t1 - t0)
            nc.vector.wait_ge(s_ld[b][c], 16)
            nc.vector.tensor_add(
                lv1[b][:, lo // 2 : lo // 2 + half],
                xts[b][:, lo : lo + half],
                xts[b][:, lo + half : lo + 2 * half],
            ).then_inc(s_dve[b], 1)
        nc.vector.wait_ge(s_rel, b + 1)
        nc.vector.tensor_copy(wide[b][:, 0:P], pss[b][:])
        nc.vector.tensor_copy(wide[b][:, P : 2 * P], pss[b][:]).then_inc(
            s_wide[b], 1
        )

    # --- PE: accumulate the 128-col blocks into the psum mean tile
    nc.tensor.wait_ge(s_w, 1)
    for b in range(BPC):
        nblk = [(CHUNKS[b][c + 1] - CHUNKS[b][c]) // 2 for c in range(4)]
        k = 0
        for c in range(4):
            nc.tensor.wait_ge(s_dve[b], c + 1)
            for _ in range(nblk[c]):
                mm = nc.tensor.matmul(
                    pss[b][:],
                    w[:],
                    lv1[b][:, 128 * k : 128 * k + 128],
                    start=(k == 0),
                    stop=(k == 7),
                )
                if k == 7:
                    mm.then_inc(s_mm, 1)
                k += 1

    nc.compile()
    return nc


def kernel(query=None, key=None, value=None, q_param=None, _trace=False):
    from concourse.bass_utils import run_bass_kernel_spmd

    global LAST_RESULT

    value = np.ascontiguousarray(np.asarray(value, dtype=np.float32))
    assert value.shape == (B, SK, DV), value.shape

    nc = _build_nc()
    shards = value.reshape(N_CORES, BPC, SK, DV)
    in_maps = [{"value": shards[i]} for i in range(N_CORES)]

    LAST_RESULT = run_bass_kernel_spmd(
        nc, in_maps, list(range(N_CORES)), trace=_trace
    )
    return np.concatenate(
        [
            np.asarray(LAST_RESULT.results[i]["out"]).astype(np.float32)
            for i in range(N_CORES)
        ],
        axis=0,
    )


# revision 8
# speedup vs baseline: 1.0530x; 1.0530x over previous
"""Trainium2 Bass kernel for nn_Attention_39676907884025.

out[b, q, :] = (1/SK) * sum_k value[b, k, :] for every q: q_param (1x1) is
broadcast over query and key, the score matrix is constant along the softmax
axis, and softmax of a constant row is exactly uniform. Only `value` touches
the device; batch B=16 is data-parallel over 8 cores (2 per core).

Raw bacc, hand-scheduled, NO nc.Block. Rationale (from perfetto traces of
the previous versions):
  - The NEFF epilogue makes every engine serially reset its fixed bank of
    ~50 semaphores (0.05-0.13 us each => 2.2-6.5 us per engine). With
    nc.Block, its exit barrier forces all engines to finish the body first,
    so the slowest reset chain lands entirely after the last store
    (~8 us of pure postamble). Emitting raw per-engine streams (no block,
    no exit barrier) lets each engine start its resets right after its own
    last instruction, overlapping them with the DMA tail. The NEFF's own
    final all-engine barrier before the loop-back jump still serializes
    executions, and the entry barrier isolates re-runs.
  - This requires bank-aware semaphore placement: an engine resets its bank
    whenever IT finishes, so a semaphore may only live in bank X if its
    last wait/increment is causally ordered before engine X's last body
    instruction. Banks: PE S[7:54], ACT S[54:105], Pool S[105:156],
    DVE S[156:207], SP S[207:256]; the user pool starts at 155 (Pool bank
    tail - burn it, Pool's body is empty and it resets almost at t=0).
  - All DMA on one HWDGE queue (SP): dma_start issue cost (~0.6 us) pays a
    SHARED HWDGE unit, so spreading across engines doesn't parallelize it,
    and a single queue still fans out over all 16 SDMA engines at full
    HBM rate while making chunk completion strictly FIFO (better
    pipelining than 2 queues' round-robin). 4 load chunks per batch
    (256 KB, 2 KB descriptors) for reduce overlap; ONE store per batch
    via a stride-0 broadcast source AP (1024 x 512 B descriptors) so the
    mean tile only needs 2 replicas in SBUF.
  - DVE pairwise-adds each chunk (128,512)f32 -> (128,256)bf16 as it
    lands; PE accumulates the 4 bf16 blocks per batch into a (128,256)
    fp32 PSUM tile with a constant 1/SK stationary (partition-reduce +
    broadcast); DVE folds psum halves -> (128,256) bf16 wide tile (two
    independent adds). ACT's only job is relaying the PE semaphore to DVE
    (only ACT may wait on PE sems - other engines hang the device).
  - Stores in bf16 (host upcasts; mean error ~0.3% << 2e-2 budget).
"""

import sys

import numpy as np

if "/opt/trn_rl_repo" not in sys.path:
    sys.path.insert(0, "/opt/trn_rl_repo")

B, SQ, SK, D, DV = 16, 2048, 2048, 128, 128
N_CORES = 8
BPC = B // N_CORES  # batches per core
P = 128

LAST_RESULT = None  # BassKernelResults of the most recent run (for profiling)


def _build_nc():
    import concourse.bacc as bacc
    import concourse.mybir as mybir

    f32 = mybir.dt.float32
    bf16 = mybir.dt.bfloat16
    nc = bacc.Bacc("TRN2", target_bir_lowering=False)

    val = nc.dram_tensor("value", [BPC, SK, DV], f32, kind="ExternalInput")
    out = nc.dram_tensor("out", [BPC, SQ, DV], bf16, kind="ExternalOutput")

    w = nc.alloc_sbuf_tensor("w_const", [P, P], bf16)
    xts = [nc.alloc_sbuf_tensor(f"xt{b}", [P, SK], f32) for b in range(BPC)]
    # pairwise sums per chunk c: bf16 at [256c, 256c+256)
    lv1 = [nc.alloc_sbuf_tensor(f"lv1_{b}", [P, 1024], bf16) for b in range(BPC)]
    # two replicas of the folded bf16 mean row
    wide = [nc.alloc_sbuf_tensor(f"wide{b}", [P, 256], bf16) for b in range(BPC)]
    pss = [nc.alloc_psum_tensor(f"ps{b}", [P, P], f32) for b in range(BPC)]

    # --- bank-aware semaphore allocation (pool pops 155, 156, ... in order)
    def sem(name, expect):
        s = nc.alloc_semaphore(name)
        assert s.num == expect, (name, s.num, expect)
        return s

    sem("dummy_pool_bank", 155)  # Pool resets S[155] at ~t=0; never use it
    # DVE bank S[156:207]: last waits/incs all causally precede DVE's last
    # fold (s_ld/s_rel: DVE's own waits; s_w/s_dve: PE consumes before its
    # stop-matmul -> s_mm -> relay -> DVE fold; s_mm: ACT consumes before
    # relay -> DVE fold).
    s_ld = [[sem(f"s_ld_{b}_{c}", 156 + 4 * b + c) for c in range(4)] for b in range(BPC)]
    s_w = sem("s_w", 164)
    s_dve = [sem(f"s_dve_{b}", 165 + b) for b in range(BPC)]
    s_mm = sem("s_mm", 167)
    s_rel = sem("s_rel", 168)
    for i in range(169, 207):  # burn the rest of the DVE bank
        sem(f"dummy_{i}", i)
    # SP bank S[207:256]: SP's own final waits consume these
    s_wide = [sem(f"s_wide_{b}", 207 + b) for b in range(BPC)]
    s_st = sem("s_st", 209)

    def xdst(b):
        return xts[b][:].rearrange("p (t d) -> p t d", d=DV)

    def xsrc(b):
        return val[b].rearrange("(p t) d -> p t d", p=P)

    # chunk boundaries in t-groups (of 16 rows): a small first chunk so the
    # SDMA engines ramp while later issues queue up, and a small last chunk
    # on batch 1 so the last-chunk -> L1 -> matmul -> store chain is short.
    # A single queue (all on SP) beats two: the 16 SDMA engines drain one
    # FIFO at ~line rate, while two queues round-robin at ~70%.
    CHUNKS = [(0, 2, 6, 10, 16), (0, 4, 8, 14, 16)]

    def load(eng, b, c):
        t0, t1 = CHUNKS[b][c], CHUNKS[b][c + 1]
        eng.dma_start(
            xdst(b)[:, t0:t1, :], xsrc(b)[:, t0:t1, :]
        ).then_inc(s_ld[b][c], 16)

    # --- SP: all loads, both stores (same FIFO), final completion wait
    for b in range(BPC):
        for c in range(4):
            load(nc.sync, b, c)
    for b in range(BPC):
        nc.sync.wait_ge(s_wide[b], 1)
        nc.sync.dma_start(
            out[b].rearrange("(p t u) d -> p t (u d)", p=P, t=8),
            wide[b][:][:, None, :].to_broadcast((P, 8, 256)),
        ).then_inc(s_st, 16)
    nc.sync.wait_ge(s_st, 32)

    # --- ACT: sole waiter on the PE semaphore; relays to DVE
    nc.scalar.wait_ge(s_mm, 1)
    nc.scalar.sem_inc(s_rel, 1)
    nc.scalar.wait_ge(s_mm, 2)
    nc.scalar.sem_inc(s_rel, 1)

    # --- DVE: L1 pairwise adds (f32 -> bf16) + psum widen into wide
    nc.vector.memset(w[:], 1.0 / SK).then_inc(s_w, 1)
    for b in range(BPC):
        for c in range(4):
            t0, t1 = CHUNKS[b][c], CHUNKS[b][c + 1]
            lo, half = 128 * t0, 64 * (t1 - t0)
            nc.vector.wait_ge(s_ld[b][c], 16)
            nc.vector.tensor_add(
                lv1[b][:, lo // 2 : lo // 2 + half],
                xts[b][:, lo : lo + half],
                xts[b][:, lo + half : lo + 2 * half],
            ).then_inc(s_dve[b], 1)
        nc.vector.wait_ge(s_rel, b + 1)
        nc.vector.tensor_copy(wide[b][:, 0:P], pss[b][:])
        nc.vector.tensor_copy(wide[b][:, P : 2 * P], pss[b][:]).then_inc(
            s_wide[b], 1
        )

    # --- PE: accumulate the 128-col blocks into the psum mean tile
    nc.tensor.wait_ge(s_w, 1)
    for b in range(BPC):
        nblk = [(CHUNKS[b][c + 1] - CHUNKS[b][c]) // 2 for c in range(4)]
        k = 0
        for c in range(4):
            nc.tensor.wait_ge(s_dve[b], c + 1)
            for _ in range(nblk[c]):
                mm = nc.tensor.matmul(
                    pss[b][:],
                    w[:],
                    lv1[b][:, 128 * k : 128 * k + 128],
                    start=(k == 0),
                    stop=(k == 7),
                )
                if k == 7:
                    mm.then_inc(s_mm, 1)
                k += 1

    nc.compile()
    return nc


def kernel(query=None, key=None, value=None, q_param=None, _trace=False):
    from concourse.bass_utils import run_bass_kernel_spmd

    global LAST_RESULT

    value = np.ascontiguousarray(np.asarray(value, dtype=np.float32))
    assert value.shape == (B, SK, DV), value.shape

    nc = _build_nc()
    shards = value.reshape(N_CORES, BPC, SK, DV)
    in_maps = [{"value": shards[i]} for i in range(N_CORES)]

    LAST_RESULT = run_bass_kernel_spmd(
        nc, in_maps, list(range(N_CORES)), trace=_trace
    )
    return np.concatenate(
        [
            np.asarray(LAST_RESULT.results[i]["out"]).astype(np.float32)
            for i in range(N_CORES)
        ],
        axis=0,
    )


# revision 11
# speedup vs baseline: 1.0909x; 1.0360x over previous
"""Trainium2 Bass kernel for nn_Attention_39676907884025.

out[b, q, :] = (1/SK) * sum_k value[b, k, :] for every q: q_param (1x1) is
broadcast over query and key, the score matrix is constant along the softmax
axis, and softmax of a constant row is exactly uniform. Only `value` touches
the device; batch B=16 is data-parallel over 8 cores (2 per core).

Raw bacc, hand-scheduled, NO nc.Block. Rationale (from perfetto traces of
the previous versions):
  - The NEFF epilogue makes every engine serially reset its fixed bank of
    ~50 semaphores (0.05-0.13 us each => 2.2-6.5 us per engine). With
    nc.Block, its exit barrier forces all engines to finish the body first,
    so the slowest reset chain lands entirely after the last store
    (~8 us of pure postamble). Emitting raw per-engine streams (no block,
    no exit barrier) lets each engine start its resets right after its own
    last instruction, overlapping them with the DMA tail. The NEFF's own
    final all-engine barrier before the loop-back jump still serializes
    executions, and the entry barrier isolates re-runs.
  - This requires bank-aware semaphore placement: an engine resets its bank
    whenever IT finishes, so a semaphore may only live in bank X if its
    last wait/increment is causally ordered before engine X's last body
    instruction. Banks: PE S[7:54], ACT S[54:105], Pool S[105:156],
    DVE S[156:207], SP S[207:256]; the user pool starts at 155 (Pool bank
    tail - burn it, Pool's body is empty and it resets almost at t=0).
  - All DMA on one HWDGE queue (SP): dma_start issue cost (~0.6 us) pays a
    SHARED HWDGE unit, so spreading across engines doesn't parallelize it,
    and a single queue still fans out over all 16 SDMA engines at full
    HBM rate while making chunk completion strictly FIFO (better
    pipelining than 2 queues' round-robin). 4 load chunks per batch
    (256 KB, 2 KB descriptors) for reduce overlap; ONE store per batch
    via a stride-0 broadcast source AP (1024 x 512 B descriptors) so the
    mean tile only needs 2 replicas in SBUF.
  - DVE pairwise-adds each chunk (128,512)f32 -> (128,256)bf16 as it
    lands; PE accumulates the 4 bf16 blocks per batch into a (128,256)
    fp32 PSUM tile with a constant 1/SK stationary (partition-reduce +
    broadcast); DVE folds psum halves -> (128,256) bf16 wide tile (two
    independent adds). ACT's only job is relaying the PE semaphore to DVE
    (only ACT may wait on PE sems - other engines hang the device).
  - Stores in bf16 (host upcasts; mean error ~0.3% << 2e-2 budget).
"""

import sys

import numpy as np

if "/opt/trn_rl_repo" not in sys.path:
    sys.path.insert(0, "/opt/trn_rl_repo")

B, SQ, SK, D, DV = 16, 2048, 2048, 128, 128
N_CORES = 8
BPC = B // N_CORES  # batches per core
P = 128

LAST_RESULT = None  # BassKernelResults of the most recent run (for profiling)


def _build_nc():
    import concourse.bacc as bacc
    import concourse.mybir as mybir

    f32 = mybir.dt.float32
    bf16 = mybir.dt.bfloat16
    nc = bacc.Bacc("TRN2", target_bir_lowering=False)

    val = nc.dram_tensor("value", [BPC, SK, DV], f32, kind="ExternalInput")
    out = nc.dram_tensor("out", [BPC, SQ, DV], bf16, kind="ExternalOutput")

    w = nc.alloc_sbuf_tensor("w_const", [P, P], bf16)
    xts = [nc.alloc_sbuf_tensor(f"xt{b}", [P, SK], f32) for b in range(BPC)]
    # pairwise sums per chunk c: bf16 at [256c, 256c+256)
    lv1 = [nc.alloc_sbuf_tensor(f"lv1_{b}", [P, 1024], bf16) for b in range(BPC)]
    # two replicas of the folded bf16 mean row
    wide = [nc.alloc_sbuf_tensor(f"wide{b}", [P, 256], bf16) for b in range(BPC)]
    pss = [nc.alloc_psum_tensor(f"ps{b}", [P, P], f32) for b in range(BPC)]

    # --- bank-aware semaphore allocation (pool pops 155, 156, ... in order)
    def sem(name, expect):
        s = nc.alloc_semaphore(name)
        assert s.num == expect, (name, s.num, expect)
        return s

    sem("dummy_pool_bank", 155)  # Pool resets S[155] at ~t=0; never use it
    # DVE bank S[156:207]: last waits/incs all causally precede DVE's last
    # fold (s_ld/s_rel: DVE's own waits; s_w/s_dve: PE consumes before its
    # stop-matmul -> s_mm -> relay -> DVE fold; s_mm: ACT consumes before
    # relay -> DVE fold).
    s_ld = [[sem(f"s_ld_{b}_{c}", 156 + 4 * b + c) for c in range(4)] for b in range(BPC)]
    s_w = sem("s_w", 164)
    s_dve = [sem(f"s_dve_{b}", 165 + b) for b in range(BPC)]
    s_mm = sem("s_mm", 167)
    s_rel = sem("s_rel", 168)
    for i in range(169, 207):  # burn the rest of the DVE bank
        sem(f"dummy_{i}", i)
    # SP bank S[207:256]: SP's own final waits consume these
    s_wide = [sem(f"s_wide_{b}", 207 + b) for b in range(BPC)]
    s_st = sem("s_st", 209)

    def xdst(b):
        return xts[b][:].rearrange("p (t d) -> p t d", d=DV)

    def xsrc(b):
        return val[b].rearrange("(p t) d -> p t d", p=P)

    # chunk boundaries in t-groups (of 16 rows): big chunks early (3-4 KB
    # descriptors keep the SDMA rings deep - 2 KB descs measured ~81% engine
    # busy), small last chunks so the last-chunk -> L1 -> matmul -> store
    # chain is short. A single queue (all on SP) beats two: the 16 SDMA
    # engines drain one FIFO at ~line rate, two queues round-robin at ~70%.
    CHUNKS = [(0, 8, 14, 16), (0, 6, 10, 14, 16)]

    def load(eng, b, c):
        t0, t1 = CHUNKS[b][c], CHUNKS[b][c + 1]
        eng.dma_start(
            xdst(b)[:, t0:t1, :], xsrc(b)[:, t0:t1, :]
        ).then_inc(s_ld[b][c], 16)

    # --- SP: all loads, both stores (same FIFO), final completion wait
    for b in range(BPC):
        for c in range(len(CHUNKS[b]) - 1):
            load(nc.sync, b, c)
    for b in range(BPC):
        nc.sync.wait_ge(s_wide[b], 1)
        nc.sync.dma_start(
            out[b].rearrange("(p t u) d -> p t (u d)", p=P, t=8),
            wide[b][:][:, None, :].to_broadcast((P, 8, 256)),
        ).then_inc(s_st, 16)
    nc.sync.wait_ge(s_st, 32)

    # --- ACT: sole waiter on the PE semaphore; relays to DVE
    nc.scalar.wait_ge(s_mm, 1)
    nc.scalar.sem_inc(s_rel, 1)
    nc.scalar.wait_ge(s_mm, 2)
    nc.scalar.sem_inc(s_rel, 1)

    # --- DVE: L1 pairwise adds (f32 -> bf16) + psum widen into wide.
    # batch-0's widen is slotted after batch-1's first L1 so it doesn't
    # stall the batch-1 chain (the relay lands around the same time).
    def l1(b, c):
        t0, t1 = CHUNKS[b][c], CHUNKS[b][c + 1]
        lo, half = 128 * t0, 64 * (t1 - t0)
        nc.vector.wait_ge(s_ld[b][c], 16)
        nc.vector.tensor_add(
            lv1[b][:, lo // 2 : lo // 2 + half],
            xts[b][:, lo : lo + half],
            xts[b][:, lo + half : lo + 2 * half],
        ).then_inc(s_dve[b], 1)

    def widen(b):
        nc.vector.wait_ge(s_rel, b + 1)
        nc.vector.tensor_copy(
            wide[b][:].rearrange("p (r d) -> p r d", r=2),
            pss[b][:][:, None, :].to_broadcast((P, 2, P)),
        ).then_inc(s_wide[b], 1)

    nc.vector.memset(w[:], 1.0 / SK).then_inc(s_w, 1)
    for c in range(len(CHUNKS[0]) - 1):
        l1(0, c)
    l1(1, 0)
    widen(0)
    for c in range(1, len(CHUNKS[1]) - 1):
        l1(1, c)
    widen(1)

    # --- PE: accumulate the 128-col blocks into the psum mean tile
    nc.tensor.wait_ge(s_w, 1)
    for b in range(BPC):
        ncnk = len(CHUNKS[b]) - 1
        nblk = [(CHUNKS[b][c + 1] - CHUNKS[b][c]) // 2 for c in range(ncnk)]
        k, total = 0, sum(nblk)
        for c in range(ncnk):
            nc.tensor.wait_ge(s_dve[b], c + 1)
            for _ in range(nblk[c]):
                mm = nc.tensor.matmul(
                    pss[b][:],
                    w[:],
                    lv1[b][:, 128 * k : 128 * k + 128],
                    start=(k == 0),
                    stop=(k == total - 1),
                )
                if k == total - 1:
                    mm.then_inc(s_mm, 1)
                k += 1

    nc.compile()
    return nc


def kernel(query=None, key=None, value=None, q_param=None, _trace=False):
    from concourse.bass_utils import run_bass_kernel_spmd

    global LAST_RESULT

    value = np.ascontiguousarray(np.asarray(value, dtype=np.float32))
    assert value.shape == (B, SK, DV), value.shape

    nc = _build_nc()
    shards = value.reshape(N_CORES, BPC, SK, DV)
    in_maps = [{"value": shards[i]} for i in range(N_CORES)]

    LAST_RESULT = run_bass_kernel_spmd(
        nc, in_maps, list(range(N_CORES)), trace=_trace
    )
    return np.concatenate(
        [
            np.asarray(LAST_RESULT.results[i]["out"]).astype(np.float32)
            for i in range(N_CORES)
        ],
        axis=0,
    )


# revision 12
# speedup vs baseline: 1.1504x; 1.0546x over previous
"""Trainium2 Bass kernel for nn_Attention_39676907884025.

out[b, q, :] = (1/SK) * sum_k value[b, k, :] for every q: q_param (1x1) is
broadcast over query and key, the score matrix is constant along the softmax
axis, and softmax of a constant row is exactly uniform. Only `value` touches
the device; batch B=16 is data-parallel over 8 cores (2 per core).

Raw bacc, hand-scheduled, NO nc.Block. Rationale (from perfetto traces of
the previous versions):
  - The NEFF epilogue makes every engine serially reset its fixed bank of
    ~50 semaphores (0.05-0.13 us each => 2.2-6.5 us per engine). With
    nc.Block, its exit barrier forces all engines to finish the body first,
    so the slowest reset chain lands entirely after the last store
    (~8 us of pure postamble). Emitting raw per-engine streams (no block,
    no exit barrier) lets each engine start its resets right after its own
    last instruction, overlapping them with the DMA tail. The NEFF's own
    final all-engine barrier before the loop-back jump still serializes
    executions, and the entry barrier isolates re-runs.
  - This requires bank-aware semaphore placement: an engine resets its bank
    whenever IT finishes, so a semaphore may only live in bank X if its
    last wait/increment is causally ordered before engine X's last body
    instruction. Banks: PE S[7:54], ACT S[54:105], Pool S[105:156],
    DVE S[156:207], SP S[207:256]; the user pool starts at 155 (Pool bank
    tail - burn it, Pool's body is empty and it resets almost at t=0).
  - All DMA on one HWDGE queue (SP): dma_start issue cost (~0.6 us) pays a
    SHARED HWDGE unit, so spreading across engines doesn't parallelize it,
    and a single queue still fans out over all 16 SDMA engines at full
    HBM rate while making chunk completion strictly FIFO (better
    pipelining than 2 queues' round-robin). 4 load chunks per batch
    (256 KB, 2 KB descriptors) for reduce overlap; ONE store per batch
    via a stride-0 broadcast source AP (1024 x 512 B descriptors) so the
    mean tile only needs 2 replicas in SBUF.
  - DVE pairwise-adds each chunk (128,512)f32 -> (128,256)bf16 as it
    lands; PE accumulates the 4 bf16 blocks per batch into a (128,256)
    fp32 PSUM tile with a constant 1/SK stationary (partition-reduce +
    broadcast); DVE folds psum halves -> (128,256) bf16 wide tile (two
    independent adds). ACT's only job is relaying the PE semaphore to DVE
    (only ACT may wait on PE sems - other engines hang the device).
  - Stores in bf16 (host upcasts; mean error ~0.3% << 2e-2 budget).
"""

import sys

import numpy as np

if "/opt/trn_rl_repo" not in sys.path:
    sys.path.insert(0, "/opt/trn_rl_repo")

B, SQ, SK, D, DV = 16, 2048, 2048, 128, 128
N_CORES = 8
BPC = B // N_CORES  # batches per core
P = 128

LAST_RESULT = None  # BassKernelResults of the most recent run (for profiling)


def _build_nc():
    import concourse.bacc as bacc
    import concourse.mybir as mybir

    f32 = mybir.dt.float32
    bf16 = mybir.dt.bfloat16
    nc = bacc.Bacc("TRN2", target_bir_lowering=False)

    val = nc.dram_tensor("value", [BPC, SK, DV], f32, kind="ExternalInput")
    out = nc.dram_tensor("out", [BPC, SQ, DV], bf16, kind="ExternalOutput")

    w = nc.alloc_sbuf_tensor("w_const", [P, P], bf16)
    xts = [nc.alloc_sbuf_tensor(f"xt{b}", [P, SK], f32) for b in range(BPC)]
    # pairwise sums per chunk c: bf16 at [256c, 256c+256)
    lv1 = [nc.alloc_sbuf_tensor(f"lv1_{b}", [P, 1024], bf16) for b in range(BPC)]
    # two replicas of the folded bf16 mean row
    wide = [nc.alloc_sbuf_tensor(f"wide{b}", [P, 256], bf16) for b in range(BPC)]
    pss = [nc.alloc_psum_tensor(f"ps{b}", [P, P], f32) for b in range(BPC)]

    # --- bank-aware semaphore allocation (pool pops 155, 156, ... in order)
    def sem(name, expect):
        s = nc.alloc_semaphore(name)
        assert s.num == expect, (name, s.num, expect)
        return s

    sem("dummy_pool_bank", 155)  # Pool resets S[155] at ~t=0; never use it
    # DVE bank S[156:207]: last waits/incs all causally precede DVE's last
    # fold (s_ld/s_rel: DVE's own waits; s_w/s_dve: PE consumes before its
    # stop-matmul -> s_mm -> relay -> DVE fold; s_mm: ACT consumes before
    # relay -> DVE fold).
    s_ld = [[sem(f"s_ld_{b}_{c}", 156 + 4 * b + c) for c in range(4)] for b in range(BPC)]
    s_w = sem("s_w", 164)
    s_dve = [sem(f"s_dve_{b}", 165 + b) for b in range(BPC)]
    s_mm = sem("s_mm", 167)
    s_rel = sem("s_rel", 168)
    for i in range(169, 207):  # burn the rest of the DVE bank
        sem(f"dummy_{i}", i)
    # SP bank S[207:256]: SP's own final waits consume these
    s_wide = [sem(f"s_wide_{b}", 207 + b) for b in range(BPC)]
    s_st = sem("s_st", 209)

    def xdst(b):
        return xts[b][:].rearrange("p (t d) -> p t d", d=DV)

    def xsrc(b):
        return val[b].rearrange("(p t) d -> p t d", p=P)

    # chunk boundaries in t-groups (of 16 rows): big chunks early (3-4 KB
    # descriptors keep the SDMA rings deep - 2 KB descs measured ~81% engine
    # busy), small last chunks so the last-chunk -> L1 -> matmul -> store
    # chain is short. A single queue (all on SP) beats two: the 16 SDMA
    # engines drain one FIFO at ~line rate, two queues round-robin at ~70%.
    CHUNKS = [(0, 16), (0, 6, 12, 14, 16)]

    def load(eng, b, c):
        t0, t1 = CHUNKS[b][c], CHUNKS[b][c + 1]
        eng.dma_start(
            xdst(b)[:, t0:t1, :], xsrc(b)[:, t0:t1, :]
        ).then_inc(s_ld[b][c], 16)

    # --- SP: all loads, both stores (same FIFO), final completion wait
    for b in range(BPC):
        for c in range(len(CHUNKS[b]) - 1):
            load(nc.sync, b, c)
    for b in range(BPC):
        nc.sync.wait_ge(s_wide[b], 1)
        nc.sync.dma_start(
            out[b].rearrange("(p t u) d -> p t (u d)", p=P, t=8),
            wide[b][:][:, None, :].to_broadcast((P, 8, 256)),
        ).then_inc(s_st, 16)
    nc.sync.wait_ge(s_st, 32)

    # --- ACT: sole waiter on the PE semaphore; relays to DVE
    nc.scalar.wait_ge(s_mm, 1)
    nc.scalar.sem_inc(s_rel, 1)
    nc.scalar.wait_ge(s_mm, 2)
    nc.scalar.sem_inc(s_rel, 1)

    # --- DVE: L1 pairwise adds (f32 -> bf16) + psum widen into wide.
    # batch-0's widen is slotted after batch-1's first L1 so it doesn't
    # stall the batch-1 chain (the relay lands around the same time).
    def l1(b, c):
        t0, t1 = CHUNKS[b][c], CHUNKS[b][c + 1]
        lo, half = 128 * t0, 64 * (t1 - t0)
        nc.vector.wait_ge(s_ld[b][c], 16)
        nc.vector.tensor_add(
            lv1[b][:, lo // 2 : lo // 2 + half],
            xts[b][:, lo : lo + half],
            xts[b][:, lo + half : lo + 2 * half],
        ).then_inc(s_dve[b], 1)

    def widen(b):
        nc.vector.wait_ge(s_rel, b + 1)
        nc.vector.tensor_copy(
            wide[b][:].rearrange("p (r d) -> p r d", r=2),
            pss[b][:][:, None, :].to_broadcast((P, 2, P)),
        ).then_inc(s_wide[b], 1)

    nc.vector.memset(w[:], 1.0 / SK).then_inc(s_w, 1)
    for c in range(len(CHUNKS[0]) - 1):
        l1(0, c)
    l1(1, 0)
    widen(0)
    for c in range(1, len(CHUNKS[1]) - 1):
        l1(1, c)
    widen(1)

    # --- PE: accumulate the 128-col blocks into the psum mean tile
    nc.tensor.wait_ge(s_w, 1)
    for b in range(BPC):
        ncnk = len(CHUNKS[b]) - 1
        nblk = [(CHUNKS[b][c + 1] - CHUNKS[b][c]) // 2 for c in range(ncnk)]
        k, total = 0, sum(nblk)
        for c in range(ncnk):
            nc.tensor.wait_ge(s_dve[b], c + 1)
            for _ in range(nblk[c]):
                mm = nc.tensor.matmul(
                    pss[b][:],
                    w[:],
                    lv1[b][:, 128 * k : 128 * k + 128],
                    start=(k == 0),
                    stop=(k == total - 1),
                )
                if k == total - 1:
                    mm.then_inc(s_mm, 1)
                k += 1

    nc.compile()
    return nc


def kernel(query=None, key=None, value=None, q_param=None, _trace=False):
    from concourse.bass_utils import run_bass_kernel_spmd

    global LAST_RESULT

    value = np.ascontiguousarray(np.asarray(value, dtype=np.float32))
    assert value.shape == (B, SK, DV), value.shape

    nc = _build_nc()
    shards = value.reshape(N_CORES, BPC, SK, DV)
    in_maps = [{"value": shards[i]} for i in range(N_CORES)]

    LAST_RESULT = run_bass_kernel_spmd(
        nc, in_maps, list(range(N_CORES)), trace=_trace
    )
    return np.concatenate(
        [
            np.asarray(LAST_RESULT.results[i]["out"]).astype(np.float32)
            for i in range(N_CORES)
        ],
        axis=0,
    )
